# revision 1
# baseline (speedup 1.0000x reference)
"""Trainium2 Bass kernel for nn_CustomLoss_69999376990919.

Math: the reference's A-inner-product modified Gram-Schmidt + projection
collapses to per-sample 4x4 Gram matrices
    G[s] = P_s diag(a_s) P_s^T,   R[s] = P_s diag(a_s) T_s
after which   loss = mean_s (4 - tr(R^T G^{-1} R)) / 4
(Cholesky of G == Gram-Schmidt in exact arithmetic; <v,Av> > 0 always holds
since coefficients > 0).  The device streams all inputs (memory-bound) and
produces G/R; the tiny 4x4 solves run on the host in float64.

Sharding: pure data parallelism, batch axis 0 split across 8 cores
(64 samples each).  Per core, samples run in 2 groups of 32 (bigger groups
amortize the ~60ns fixed cost of the per-matmul weight load best).
Layout: n = p*128 + f (p = SBUF partition, f = free chunk).  Per f-chunk,
a bf16 matmul pair accumulates G and R for all 32 samples jointly:
  lhsT = W(f) = (a*P)(f) as [128, (i,s)] stationary,
  rhs  = P(f) / T(f) as [128, (s,j)] moving,
  PSUM[(i,s), (s',j)] accumulated over the 128 f-chunks; the s==s' block
diagonals are the per-sample G/R entries (extracted on host).
Per group the DMAs are ordered p, a, t (SWDGE is FIFO) and the matmuls run
as a G-phase then an R-phase in t-half chunks, so compute starts as soon as
the group's predictions have landed and only the final R half-phase is
exposed after the last DMA.  bf16 is safe: the loss is 1 - O(1e-4);
bf16-quantized inputs move the final scalar by ~1e-9 relative.
"""

import os
from contextlib import ExitStack

import numpy as np

import concourse.bacc as bacc
import concourse.bass as bass
import concourse.tile as tile
from concourse import mybir
from concourse.bass_utils import run_bass_kernel_spmd

B, C, N = 512, 4, 16384
H = 0.0078125  # grid spacing; A = diag(h^2 * coefficients)
NCORES = 8
SPC = B // NCORES  # 64 samples per core
GS = 32            # samples per group
NG = SPC // GS     # 2 groups per core
P = 128            # SBUF partitions; n = p*128 + f
F = N // P         # 128 f-chunks
FH = F // 2        # f-half (t16/w16 tile granularity)
QP = C * GS        # psum partitions (i, s)

_CACHE = {}


def _build_bass():
    nc = bacc.Bacc(trn_type="TRN2")
    coeff = nc.dram_tensor("coeff", [SPC, N], mybir.dt.float32, kind="ExternalInput")
    preds = nc.dram_tensor("preds", [SPC, C, N], mybir.dt.float32, kind="ExternalInput")
    targs = nc.dram_tensor("targs", [SPC, N, C], mybir.dt.float32, kind="ExternalInput")
    out = nc.dram_tensor(
        "gr_out", [QP, NG * 2 * C * GS], mybir.dt.float32, kind="ExternalOutput"
    )

    coeff_v = coeff[:].rearrange("s (p f) -> p s f", p=P)
    preds_v = preds[:].rearrange("s j (p f) -> p s j f", p=P)
    targs_v = targs[:].rearrange("s (p f) m -> p s f m", p=P)

    with tile.TileContext(nc) as tc, ExitStack() as ctx:
        p32s = ctx.enter_context(tc.tile_pool(name="p32s", bufs=2))
        p16s = ctx.enter_context(tc.tile_pool(name="p16s", bufs=2))
        t16s = ctx.enter_context(tc.tile_pool(name="t16s", bufs=6))
        a16s = ctx.enter_context(tc.tile_pool(name="a16s", bufs=2))
        w16s = ctx.enter_context(tc.tile_pool(name="w16s", bufs=2))
        outs = ctx.enter_context(tc.tile_pool(name="outs", bufs=1))
        psums = ctx.enter_context(tc.tile_pool(name="psums", bufs=2, space="PSUM"))

        out_stage = outs.tile([QP, NG * 2 * C * GS], mybir.dt.float32)

        GA = 12       # p-samples on the SWDGE cast queue (it also carries a16
        SE = 4        # and pays a cast-rate derate; HWDGE takes the rest)

        for g in range(NG):
            sl = slice(g * GS, (g + 1) * GS)

            # coefficients first on SWDGE: tiny, and they gate the W multiply
            a16 = a16s.tile([P, GS, F], mybir.dt.bfloat16, tag="a16")
            nc.gpsimd.dma_start(out=a16[:], in_=coeff_v[:, sl, :])

            # predictions gate W and the G-phase, so split them across BOTH
            # DMA queues (the SDMA engines round-robin between queues, so a
            # single-queue load only gets ~half the bandwidth while the other
            # queue has work): 12 samples as a SWDGE cast-DMA, 20 as fp32
            # HWDGE loads converted to bf16 by the otherwise-idle ScalarE
            p16 = p16s.tile([P, GS, C, F], mybir.dt.bfloat16, tag="p16")
            sx = slice(g * GS, g * GS + GA)
            nc.gpsimd.dma_start(out=p16[:, 0:GA, :, :], in_=preds_v[:, sx, :, :])
            for x in range(5):
                s0 = g * GS + GA + x * SE
                p32x = p32s.tile([P, SE, C, F], mybir.dt.float32, tag="p32")
                nc.sync.dma_start(out=p32x[:], in_=preds_v[:, s0 : s0 + SE, :, :])
                nc.scalar.copy(
                    out=p16[:, GA + x * SE : GA + (x + 1) * SE, :, :], in_=p32x[:]
                )

            t16 = []
            for h in range(4):
                th = t16s.tile(
                    [P, GS, F // 4, C], mybir.dt.bfloat16, tag="t16",
                    name=f"t16_{g}_{h}",
                )
                nc.gpsimd.dma_start(
                    out=th[:],
                    in_=targs_v[:, sl, h * (F // 4) : (h + 1) * (F // 4), :],
                )
                t16.append(th)

            # W = a * p in bf16, layout [P, i, s, f]: f-contiguous DVE writes
            w16 = []
            for h in range(2):
                wh = w16s.tile(
                    [P, C, GS, FH], mybir.dt.bfloat16, tag="w16", name=f"w16_{g}_{h}"
                )
                for i in range(C):
                    nc.vector.tensor_mul(
                        wh[:, i, :, :],
                        a16[:, :, h * FH : (h + 1) * FH],
                        p16[:, :, i, h * FH : (h + 1) * FH],
                    )
                w16.append(wh)

            psum_g = psums.tile([QP, GS * C], mybir.dt.float32, tag="pg")
            psum_r = psums.tile([QP, GS * C], mybir.dt.float32, tag="pr")

            # G-phase: only needs p16 + W
            for f in range(F):
                h, fl = divmod(f, FH)
                nc.tensor.matmul(
                    psum_g[:],
                    w16[h][:, :, :, fl],   # [128, (i, s)] stationary
                    p16[:, :, :, f],       # [128, (s, j)] moving
                    start=(f == 0),
                    stop=(f == F - 1),
                )
            # R-phase: chases the four t16 quarter tiles
            for f in range(F):
                h, fl = divmod(f, FH)
                q, fq = divmod(f, F // 4)
                nc.tensor.matmul(
                    psum_r[:],
                    w16[h][:, :, :, fl],
                    t16[q][:, :, fq, :],   # [128, (s, m)] moving
                    start=(f == 0),
                    stop=(f == F - 1),
                )

            gw = 2 * C * GS  # out_stage columns per group
            nc.scalar.copy(
                out=out_stage[:, g * gw : g * gw + C * GS], in_=psum_g[:]
            )
            nc.scalar.copy(
                out=out_stage[:, g * gw + C * GS : (g + 1) * gw], in_=psum_r[:]
            )
            # drain this group's results while the next group computes
            nc.sync.dma_start(
                out=out[:, g * gw : (g + 1) * gw],
                in_=out_stage[:, g * gw : (g + 1) * gw],
            )

    if not nc.is_finalized():
        nc.finalize()
    return nc


def _get_nc():
    if "nc" not in _CACHE:
        _CACHE["nc"] = _build_bass()
    return _CACHE["nc"]


def kernel(coefficients, predictions, targets):
    co = np.ascontiguousarray(np.asarray(coefficients, dtype=np.float32))
    pr = np.ascontiguousarray(np.asarray(predictions, dtype=np.float32))
    tg = np.ascontiguousarray(np.asarray(targets, dtype=np.float32))
    assert co.shape == (B, N) and pr.shape == (B, C, N) and tg.shape == (B, N, C)

    nc = _get_nc()
    in_maps = []
    for c in range(NCORES):
        sl = slice(c * SPC, (c + 1) * SPC)
        in_maps.append({"coeff": co[sl], "preds": pr[sl], "targs": tg[sl]})

    res = run_bass_kernel_spmd(nc, in_maps, core_ids=list(range(NCORES)))
    _CACHE["last"] = res

    # host epilogue: extract per-sample 4x4 G/R block diagonals, fp64 solve
    G = np.empty((B, C, C), np.float64)
    R = np.empty((B, C, C), np.float64)
    gw = 2 * C * GS
    for c in range(NCORES):
        o = np.asarray(res.results[c]["gr_out"], dtype=np.float64)
        for g in range(NG):
            bg = o[:, g * gw : g * gw + C * GS].reshape(C, GS, GS, C)
            br = o[:, g * gw + C * GS : (g + 1) * gw].reshape(C, GS, GS, C)
            s0 = c * SPC + g * GS
            G[s0 : s0 + GS] = np.einsum("issj->sij", bg)
            R[s0 : s0 + GS] = np.einsum("issm->sim", br)

    G = 0.5 * (G + np.swapaxes(G, 1, 2))
    Xs = np.linalg.solve(G, R)
    val = (H * H) * np.einsum("bim,bim->b", R, Xs)
    loss = np.mean((4.0 - val) / 4.0)
    return np.float32(loss)



# revision 2
# speedup vs baseline: 1.2277x; 1.2277x over previous
"""Trainium2 Bass kernel for nn_CustomLoss_69999376990919.

Math: the reference's A-inner-product modified Gram-Schmidt + projection
collapses to per-sample 4x4 Gram matrices
    G[s] = P_s diag(a_s) P_s^T,   R[s] = P_s diag(a_s) T_s
after which   loss = mean_s (4 - h^2 tr(R^T G^{-1} R)) / 4
(Cholesky of G == Gram-Schmidt in exact arithmetic; <v,Av> > 0 always holds
since coefficients > 0).  The device streams all inputs (memory-bound) and
produces G/R; the tiny 4x4 solves run on the host in float64.

Sharding: pure data parallelism, batch axis 0 split across 8 cores
(64 samples each), processed as 2 groups of 32 (PSUM block = 4*32 = 128).

Layout strategy (the whole game is DMA packet size + matmul operand
contiguity; the per-core floor is the HBM read stream 37.75 MB @ ~358 GB/s
~= 105 us, and the SBUF-AXI write-side budget ~218 GB/s shared with the
sibling NeuronCore):
  - The host pre-permutes each core's slab (pure fp32 layout change; the
    device still reads every input byte from HBM) to
        preds  [p=128, g=2, f=128, j=4, s=32]   (n = p*128 + f)
        targs  [p=128, g=2, f=128, s=32, m=4]
        coeff  [p=128, g=2, f=128, s=32]
    so every DMA reads multi-KB contiguous runs per partition (4 KB packets
    at SDMA line rate instead of 512 B runs at ~40 ns/packet) and every
    matmul operand slice [:, f, :, :] is contiguous in SBUF.
  - All loads are SWDGE cast-DMAs fp32->bf16 (>=2 KB writes, no sub-512 B
    read-modify-write), halving the SBUF-AXI write-side bytes: ~87 us.
  - DVE computes W = a (.) P into [128, f, j, s] with contiguous reads and
    writes (2x bf16 perf mode eligible), ~17-35 us.
  - TensorE per f: LDW(W[f]) (contiguous -> FWL) + MM psum_g += W[f]^T P[f]
    + MM psum_r += W[f]^T T[f], both moving operands contiguous 128-column
    slices (~56-81 ns/MM instead of 444 ns for strided columns): ~35-45 us.
  Everything hides under the ~105-111 us HBM stream.
bf16 is safe: the loss is 1 - O(1e-4); quantization moves it by ~1e-9 rel.
"""

import numpy as np

import concourse.bacc as bacc
from contextlib import ExitStack

import concourse.tile as tile
from concourse import mybir
from concourse.bass_utils import run_bass_kernel_spmd

B, C, N = 512, 4, 16384
H = 0.0078125  # grid spacing; A = diag(h^2 * coefficients)
NCORES = 8
SPC = B // NCORES  # 64 samples per core
NG = 2             # groups per core
GS = SPC // NG     # 32 samples per group
P = 128            # SBUF partitions; n = p*128 + f
F = N // P         # 128 f-steps
FC = 4             # f-chunks per group
FCL = F // FC      # 32 f-steps per chunk

_CACHE = {}


def _build_bass():
    nc = bacc.Bacc(trn_type="TRN2")
    coeff = nc.dram_tensor("coeff", [P, NG * F * GS], mybir.dt.float32,
                           kind="ExternalInput")
    preds = nc.dram_tensor("preds", [P, NG * F * C * GS], mybir.dt.float32,
                           kind="ExternalInput")
    targs = nc.dram_tensor("targs", [P, NG * F * GS * C], mybir.dt.float32,
                           kind="ExternalInput")
    out = nc.dram_tensor("gr_out", [P, NG * 2 * C * GS], mybir.dt.float32,
                         kind="ExternalOutput")

    coeff_v = coeff[:].rearrange("p (g f s) -> p g f s", g=NG, f=F)
    preds_v = preds[:].rearrange("p (g f j s) -> p g f j s", g=NG, f=F, j=C)
    targs_v = targs[:].rearrange("p (g f s m) -> p g f s m", g=NG, f=F, s=GS)

    with tile.TileContext(nc) as tc, ExitStack() as ctx:
        a_pool = ctx.enter_context(tc.tile_pool(name="a_pool", bufs=4))
        p_pool = ctx.enter_context(tc.tile_pool(name="p_pool", bufs=4))
        t_pool = ctx.enter_context(tc.tile_pool(name="t_pool", bufs=4))
        w_pool = ctx.enter_context(tc.tile_pool(name="w_pool", bufs=2))
        outs = ctx.enter_context(tc.tile_pool(name="outs", bufs=1))
        psums = ctx.enter_context(tc.tile_pool(name="psums", bufs=4, space="PSUM"))

        out_stage = outs.tile([P, NG * 2 * C * GS], mybir.dt.float32)

        for g in range(NG):
            w16 = w_pool.tile([P, F, C, GS], mybir.dt.bfloat16, tag="w16",
                              name=f"w16_{g}")
            psum_g = psums.tile([P, C * GS], mybir.dt.float32, tag="pg",
                                name=f"pg_{g}")
            psum_r = psums.tile([P, C * GS], mybir.dt.float32, tag="pr",
                                name=f"pr_{g}")

            for fc in range(FC):
                fsl = slice(fc * FCL, (fc + 1) * FCL)
                a16 = a_pool.tile([P, FCL, GS], mybir.dt.bfloat16, tag="a16",
                                  name=f"a16_{g}_{fc}")
                p16 = p_pool.tile([P, FCL, C, GS], mybir.dt.bfloat16, tag="p16",
                                  name=f"p16_{g}_{fc}")
                t16 = t_pool.tile([P, FCL, GS, C], mybir.dt.bfloat16, tag="t16",
                                  name=f"t16_{g}_{fc}")
                nc.gpsimd.dma_start(out=a16[:], in_=coeff_v[:, g, fsl, :])
                nc.gpsimd.dma_start(out=p16[:], in_=preds_v[:, g, fsl, :, :])
                nc.gpsimd.dma_start(out=t16[:], in_=targs_v[:, g, fsl, :, :])

                # W = a * p, all APs contiguous per j (a broadcast by loop)
                for j in range(C):
                    nc.vector.tensor_mul(
                        w16[:, fsl, j, :], a16[:], p16[:, :, j, :]
                    )

                for fl in range(FCL):
                    f = fc * FCL + fl
                    nc.tensor.matmul(
                        psum_g[:],
                        w16[:, f, :, :],     # stationary [128, (j,s)] contiguous
                        p16[:, fl, :, :],    # moving [128, (j',s')] contiguous
                        start=(f == 0),
                        stop=(f == F - 1),
                    )
                    nc.tensor.matmul(
                        psum_r[:],
                        w16[:, f, :, :],
                        t16[:, fl, :, :],    # moving [128, (s',m)] contiguous
                        start=(f == 0),
                        stop=(f == F - 1),
                    )

            gw = 2 * C * GS
            nc.scalar.copy(out=out_stage[:, g * gw : g * gw + C * GS],
                           in_=psum_g[:])
            nc.scalar.copy(out=out_stage[:, g * gw + C * GS : (g + 1) * gw],
                           in_=psum_r[:])
            nc.sync.dma_start(
                out=out[:, g * gw : (g + 1) * gw],
                in_=out_stage[:, g * gw : (g + 1) * gw],
            )

    if not nc.is_finalized():
        nc.finalize()
    return nc


def _get_nc():
    if "nc" not in _CACHE:
        _CACHE["nc"] = _build_bass()
    return _CACHE["nc"]


def kernel(coefficients, predictions, targets):
    co = np.asarray(coefficients, dtype=np.float32)
    pr = np.asarray(predictions, dtype=np.float32)
    tg = np.asarray(targets, dtype=np.float32)
    assert co.shape == (B, N) and pr.shape == (B, C, N) and tg.shape == (B, N, C)

    # Host-side pure permutation into DMA/matmul-friendly layouts (fp32;
    # the device still streams every byte).  c=core, g=group, s=sample in
    # group, p=partition (n div 128), f=n mod ... (n = p*128 + f).
    co_p = np.ascontiguousarray(
        co.reshape(NCORES, NG, GS, P, F).transpose(0, 3, 1, 4, 2)
    )  # [c, p, g, f, s]
    pr_p = np.ascontiguousarray(
        pr.reshape(NCORES, NG, GS, C, P, F).transpose(0, 4, 1, 5, 3, 2)
    )  # [c, p, g, f, j, s]
    tg_p = np.ascontiguousarray(
        tg.reshape(NCORES, NG, GS, P, F, C).transpose(0, 3, 1, 4, 2, 5)
    )  # [c, p, g, f, s, m]

    nc = _get_nc()
    in_maps = []
    for c in range(NCORES):
        in_maps.append({
            "coeff": co_p[c].reshape(P, NG * F * GS),
            "preds": pr_p[c].reshape(P, NG * F * C * GS),
            "targs": tg_p[c].reshape(P, NG * F * GS * C),
        })

    res = run_bass_kernel_spmd(nc, in_maps, core_ids=list(range(NCORES)))
    _CACHE["last"] = res

    # host epilogue: extract per-sample 4x4 G/R diagonals, fp64 solve
    G = np.empty((B, C, C), np.float64)
    R = np.empty((B, C, C), np.float64)
    gw = 2 * C * GS
    for c in range(NCORES):
        o = np.asarray(res.results[c]["gr_out"], dtype=np.float64)
        for g in range(NG):
            bg = o[:, g * gw : g * gw + C * GS].reshape(C, GS, C, GS)
            br = o[:, g * gw + C * GS : (g + 1) * gw].reshape(C, GS, GS, C)
            s0 = c * SPC + g * GS
            G[s0 : s0 + GS] = np.einsum("isjs->sij", bg)
            R[s0 : s0 + GS] = np.einsum("issm->sim", br)

    G = 0.5 * (G + np.swapaxes(G, 1, 2))
    Xs = np.linalg.solve(G, R)
    val = (H * H) * np.einsum("bim,bim->b", R, Xs)
    loss = np.mean((4.0 - val) / 4.0)
    return np.float32(loss)


# revision 3
# speedup vs baseline: 1.2288x; 1.0009x over previous
"""Trainium2 Bass kernel for nn_CustomLoss_69999376990919.

Math: the reference's A-inner-product modified Gram-Schmidt + projection
collapses to per-sample 4x4 Gram matrices
    G[s] = P_s diag(a_s) P_s^T,   R[s] = P_s diag(a_s) T_s
after which   loss = mean_s (4 - h^2 tr(R^T G^{-1} R)) / 4
(Cholesky of G == Gram-Schmidt in exact arithmetic; <v,Av> > 0 always holds
since coefficients > 0).  The device streams all inputs (memory-bound) and
produces G/R; the tiny 4x4 solves run on the host in float64.

Sharding: pure data parallelism, batch axis 0 split across 8 cores
(64 samples each), processed as 2 groups of 32 (PSUM block = 4*32 = 128).

Layout strategy (the knobs are DMA packet size, SBUF-AXI write bytes, and
matmul operand contiguity; the per-core floor is the HBM read stream
37.75 MB @ ~358 GB/s ~= 105 us):
  - The host pre-permutes each core's slab (pure fp32 layout change; the
    device still reads every input byte from HBM) to
        pt     [p=128, g=2, f=128, 256]   (n = p*128 + f) where the 256
               columns are preds (j,s) then targs (s,m) interleaved per f
        coeff  [p=128, g=2, f=128, s=32]
    so every DMA reads multi-KB contiguous runs per partition (4 KB packets
    at SDMA line rate instead of 512 B runs at ~40 ns/packet) and every
    matmul operand slice is a flat contiguous 2-D AP.
  - All loads are SWDGE cast-DMAs fp32->bf16 (>=1 KB writes, no sub-512 B
    read-modify-write), halving the SBUF-AXI write-side bytes: ~87 us.
  - DVE computes W = a (.) P into a flat [128, f, (j s)] tile with
    contiguous reads and writes (2x bf16 perf mode eligible).
  - TensorE per f: one LDW(W[f]) (flat 128-col bf16 -> FWL) + ONE matmul
    with the 256-column moving slice pt[f] accumulating G and R blocks
    side by side in PSUM: 256 matmuls total, burst ~2-3 us per 16-f chunk
    against a ~6.8 us/chunk DMA cadence, so the PE never idles past the
    HAM MID window and stays at K=8/8.
  Everything hides under the ~105-111 us HBM stream.
bf16 is safe: the loss is 1 - O(1e-4); quantization moves it by ~1e-9 rel.
"""

import numpy as np

import concourse.bacc as bacc
from contextlib import ExitStack

import concourse.tile as tile
from concourse import mybir
from concourse.bass_utils import run_bass_kernel_spmd

B, C, N = 512, 4, 16384
H = 0.0078125  # grid spacing; A = diag(h^2 * coefficients)
NCORES = 8
SPC = B // NCORES  # 64 samples per core
NG = 2             # groups per core
GS = SPC // NG     # 32 samples per group
P = 128            # SBUF partitions; n = p*128 + f
F = N // P         # 128 f-steps
FC = 8             # f-chunks per group
FCL = F // FC      # 16 f-steps per chunk
X = 2 * C * GS     # 256 moving columns: preds (j,s) ++ targs (s,m)

_CACHE = {}


def _build_bass():
    nc = bacc.Bacc(trn_type="TRN2")
    coeff = nc.dram_tensor("coeff", [P, NG * F * GS], mybir.dt.float32,
                           kind="ExternalInput")
    ptin = nc.dram_tensor("ptin", [P, NG * F * X], mybir.dt.float32,
                          kind="ExternalInput")
    out = nc.dram_tensor("gr_out", [P, NG * X], mybir.dt.float32,
                         kind="ExternalOutput")

    coeff_v = coeff[:].rearrange("p (g f s) -> p g f s", g=NG, f=F)
    pt_v = ptin[:].rearrange("p (g f x) -> p g f x", g=NG, f=F)

    with tile.TileContext(nc) as tc, ExitStack() as ctx:
        a_pool = ctx.enter_context(tc.tile_pool(name="a_pool", bufs=4))
        pt_pool = ctx.enter_context(tc.tile_pool(name="pt_pool", bufs=4))
        w_pool = ctx.enter_context(tc.tile_pool(name="w_pool", bufs=2))
        outs = ctx.enter_context(tc.tile_pool(name="outs", bufs=1))
        psums = ctx.enter_context(tc.tile_pool(name="psums", bufs=2, space="PSUM"))

        out_stage = outs.tile([P, NG * X], mybir.dt.float32)

        for g in range(NG):
            w16 = w_pool.tile([P, F, C * GS], mybir.dt.bfloat16, tag="w16",
                              name=f"w16_{g}")
            psum = psums.tile([P, X], mybir.dt.float32, tag="ps",
                              name=f"ps_{g}")

            for fc in range(FC):
                fsl = slice(fc * FCL, (fc + 1) * FCL)
                a16 = a_pool.tile([P, FCL, GS], mybir.dt.bfloat16, tag="a16",
                                  name=f"a16_{g}_{fc}")
                pt16 = pt_pool.tile([P, FCL, X], mybir.dt.bfloat16, tag="pt16",
                                    name=f"pt16_{g}_{fc}")
                nc.gpsimd.dma_start(out=a16[:], in_=coeff_v[:, g, fsl, :])
                nc.gpsimd.dma_start(out=pt16[:], in_=pt_v[:, g, fsl, :])

                # W = a * p, all APs contiguous per j (a broadcast by loop)
                for j in range(C):
                    nc.vector.tensor_mul(
                        w16[:, fsl, j * GS : (j + 1) * GS],
                        a16[:],
                        pt16[:, :, j * GS : (j + 1) * GS],
                    )

                for fl in range(FCL):
                    f = fc * FCL + fl
                    nc.tensor.matmul(
                        psum[:],
                        w16[:, f, :],     # stationary [128, 128] contiguous
                        pt16[:, fl, :],   # moving [128, 256] contiguous
                        start=(f == 0),
                        stop=(f == F - 1),
                    )

            nc.scalar.copy(out=out_stage[:, g * X : (g + 1) * X], in_=psum[:])
            nc.sync.dma_start(
                out=out[:, g * X : (g + 1) * X],
                in_=out_stage[:, g * X : (g + 1) * X],
            )

    if not nc.is_finalized():
        nc.finalize()
    return nc


def _get_nc():
    if "nc" not in _CACHE:
        _CACHE["nc"] = _build_bass()
    return _CACHE["nc"]


def kernel(coefficients, predictions, targets):
    co = np.asarray(coefficients, dtype=np.float32)
    pr = np.asarray(predictions, dtype=np.float32)
    tg = np.asarray(targets, dtype=np.float32)
    assert co.shape == (B, N) and pr.shape == (B, C, N) and tg.shape == (B, N, C)

    # Host-side pure permutation into DMA/matmul-friendly layouts (fp32;
    # the device still streams every byte).  c=core, g=group, s=sample in
    # group, p=partition, f (n = p*128 + f), j=class, m=target column.
    co_p = np.ascontiguousarray(
        co.reshape(NCORES, NG, GS, P, F).transpose(0, 3, 1, 4, 2)
    )  # [c, p, g, f, s]
    pr_p = pr.reshape(NCORES, NG, GS, C, P, F).transpose(0, 4, 1, 5, 3, 2)
    tg_p = tg.reshape(NCORES, NG, GS, P, F, C).transpose(0, 3, 1, 4, 2, 5)
    pt = np.concatenate(
        [pr_p.reshape(NCORES, P, NG, F, C * GS),
         tg_p.reshape(NCORES, P, NG, F, GS * C)],
        axis=-1,
    )  # [c, p, g, f, 256]
    pt = np.ascontiguousarray(pt)

    nc = _get_nc()
    in_maps = []
    for c in range(NCORES):
        in_maps.append({
            "coeff": co_p[c].reshape(P, NG * F * GS),
            "ptin": pt[c].reshape(P, NG * F * X),
        })

    res = run_bass_kernel_spmd(nc, in_maps, core_ids=list(range(NCORES)))
    _CACHE["last"] = res

    # host epilogue: extract per-sample 4x4 G/R diagonals, fp64 solve
    G = np.empty((B, C, C), np.float64)
    R = np.empty((B, C, C), np.float64)
    for c in range(NCORES):
        o = np.asarray(res.results[c]["gr_out"], dtype=np.float64)
        for g in range(NG):
            bg = o[:, g * X : g * X + C * GS].reshape(C, GS, C, GS)
            br = o[:, g * X + C * GS : (g + 1) * X].reshape(C, GS, GS, C)
            s0 = c * SPC + g * GS
            G[s0 : s0 + GS] = np.einsum("isjs->sij", bg)
            R[s0 : s0 + GS] = np.einsum("issm->sim", br)

    G = 0.5 * (G + np.swapaxes(G, 1, 2))
    Xs = np.linalg.solve(G, R)
    val = (H * H) * np.einsum("bim,bim->b", R, Xs)
    loss = np.mean((4.0 - val) / 4.0)
    return np.float32(loss)


# revision 4
# speedup vs baseline: 1.4224x; 1.1576x over previous
"""Trainium2 Bass kernel for nn_CustomLoss_69999376990919.

Math: the reference's A-inner-product modified Gram-Schmidt + projection
collapses to per-sample 4x4 Gram matrices
    G[s] = P_s diag(a_s) P_s^T,   R[s] = P_s diag(a_s) T_s
after which   loss = mean_s (4 - h^2 tr(R^T G^{-1} R)) / 4
(Cholesky of G == Gram-Schmidt in exact arithmetic; <v,Av> > 0 always holds
since coefficients > 0).  The device streams all inputs (memory-bound) and
produces G/R; the tiny 4x4 solves run on the host in float64.

Sharding: pure data parallelism, batch axis 0 split across 8 cores
(64 samples each), processed as 2 groups of 32 (PSUM block = 4*32 = 128).

Layout strategy (the knobs are DMA packet size, SBUF-AXI write bytes, and
matmul operand contiguity; the per-core floor is the HBM read stream
37.75 MB @ ~358 GB/s ~= 105 us):
  - The host pre-permutes each core's slab (pure fp32 layout change; the
    device still reads every input byte from HBM) to
        pt     [p=128, g=2, f=128, 256]   (n = p*128 + f) where the 256
               columns are preds (j,s) then targs (s,m) interleaved per f
        coeff  [p=128, g=2, f=128, s=32]
    so every DMA reads multi-KB contiguous runs per partition (4 KB packets
    at SDMA line rate instead of 512 B runs at ~40 ns/packet) and every
    matmul operand slice is a flat contiguous 2-D AP.
  - All loads are SWDGE cast-DMAs fp32->bf16 (>=1 KB writes, no sub-512 B
    read-modify-write), halving the SBUF-AXI write-side bytes: ~87 us.
  - DVE computes W = a (.) P into a flat [128, f, (j s)] tile with
    contiguous reads and writes (2x bf16 perf mode eligible).
  - TensorE per f: one LDW(W[f]) (flat 128-col bf16 -> FWL) + ONE matmul
    with the 256-column moving slice pt[f] accumulating G and R blocks
    side by side in PSUM: 256 matmuls total, burst ~2-3 us per 16-f chunk
    against a ~6.8 us/chunk DMA cadence, so the PE never idles past the
    HAM MID window and stays at K=8/8.
  Everything hides under the ~105-111 us HBM stream.
bf16 is safe: the loss is 1 - O(1e-4); quantization moves it by ~1e-9 rel.
"""

import numpy as np

import concourse.bacc as bacc
from contextlib import ExitStack

import concourse.tile as tile
from concourse import mybir
from concourse.bass_utils import run_bass_kernel_spmd

B, C, N = 512, 4, 16384
H = 0.0078125  # grid spacing; A = diag(h^2 * coefficients)
NCORES = 8
SPC = B // NCORES  # 64 samples per core
NG = 2             # groups per core
GS = SPC // NG     # 32 samples per group
P = 128            # SBUF partitions; n = p*128 + f
F = N // P         # 128 f-steps
FC = 8             # f-chunks per group
FCL = F // FC      # 16 f-steps per chunk
X = 2 * C * GS     # 256 moving columns: preds (j,s) ++ targs (s,m)

_CACHE = {}


def _build_bass():
    nc = bacc.Bacc(trn_type="TRN2")
    coeff = nc.dram_tensor("coeff", [P, NG * F * GS], mybir.dt.float32,
                           kind="ExternalInput")
    ptin = nc.dram_tensor("ptin", [P, NG * F * X], mybir.dt.float32,
                          kind="ExternalInput")
    out = nc.dram_tensor("gr_out", [P, NG * X], mybir.dt.float32,
                         kind="ExternalOutput")

    coeff_v = coeff[:].rearrange("p (g f s) -> p g f s", g=NG, f=F)
    pt_v = ptin[:].rearrange("p (g f x) -> p g f x", g=NG, f=F)

    with tile.TileContext(nc) as tc, ExitStack() as ctx:
        a_pool = ctx.enter_context(tc.tile_pool(name="a_pool", bufs=8))
        pt_pool = ctx.enter_context(tc.tile_pool(name="pt_pool", bufs=8))
        w_pool = ctx.enter_context(tc.tile_pool(name="w_pool", bufs=2))
        outs = ctx.enter_context(tc.tile_pool(name="outs", bufs=1))
        psums = ctx.enter_context(tc.tile_pool(name="psums", bufs=2, space="PSUM"))

        out_stage = outs.tile([P, NG * X], mybir.dt.float32)

        for g in range(NG):
            w16 = w_pool.tile([P, F, C * GS], mybir.dt.bfloat16, tag="w16",
                              name=f"w16_{g}")
            psum = psums.tile([P, X], mybir.dt.float32, tag="ps",
                              name=f"ps_{g}")

            for fc in range(FC):
                fsl = slice(fc * FCL, (fc + 1) * FCL)
                a16 = a_pool.tile([P, FCL, GS], mybir.dt.bfloat16, tag="a16",
                                  name=f"a16_{g}_{fc}")
                pt16 = pt_pool.tile([P, FCL, X], mybir.dt.bfloat16, tag="pt16",
                                    name=f"pt16_{g}_{fc}")
                nc.gpsimd.dma_start(out=a16[:], in_=coeff_v[:, g, fsl, :])
                nc.gpsimd.dma_start(out=pt16[:], in_=pt_v[:, g, fsl, :])

                # W = a * p, all APs contiguous per j (a broadcast by loop)
                for j in range(C):
                    nc.vector.tensor_mul(
                        w16[:, fsl, j * GS : (j + 1) * GS],
                        a16[:],
                        pt16[:, :, j * GS : (j + 1) * GS],
                    )

                for fl in range(FCL):
                    f = fc * FCL + fl
                    nc.tensor.matmul(
                        psum[:],
                        w16[:, f, :],     # stationary [128, 128] contiguous
                        pt16[:, fl, :],   # moving [128, 256] contiguous
                        start=(f == 0),
                        stop=(f == F - 1),
                    )

            nc.scalar.copy(out=out_stage[:, g * X : (g + 1) * X], in_=psum[:])
            nc.sync.dma_start(
                out=out[:, g * X : (g + 1) * X],
                in_=out_stage[:, g * X : (g + 1) * X],
            )

    if not nc.is_finalized():
        nc.finalize()
    return nc


def _get_nc():
    if "nc" not in _CACHE:
        _CACHE["nc"] = _build_bass()
    return _CACHE["nc"]


def kernel(coefficients, predictions, targets):
    co = np.asarray(coefficients, dtype=np.float32)
    pr = np.asarray(predictions, dtype=np.float32)
    tg = np.asarray(targets, dtype=np.float32)
    assert co.shape == (B, N) and pr.shape == (B, C, N) and tg.shape == (B, N, C)

    # Host-side pure permutation into DMA/matmul-friendly layouts (fp32;
    # the device still streams every byte).  c=core, g=group, s=sample in
    # group, p=partition, f (n = p*128 + f), j=class, m=target column.
    co_p = np.ascontiguousarray(
        co.reshape(NCORES, NG, GS, P, F).transpose(0, 3, 1, 4, 2)
    )  # [c, p, g, f, s]
    pr_p = pr.reshape(NCORES, NG, GS, C, P, F).transpose(0, 4, 1, 5, 3, 2)
    tg_p = tg.reshape(NCORES, NG, GS, P, F, C).transpose(0, 3, 1, 4, 2, 5)
    pt = np.concatenate(
        [pr_p.reshape(NCORES, P, NG, F, C * GS),
         tg_p.reshape(NCORES, P, NG, F, GS * C)],
        axis=-1,
    )  # [c, p, g, f, 256]
    pt = np.ascontiguousarray(pt)

    nc = _get_nc()
    in_maps = []
    for c in range(NCORES):
        in_maps.append({
            "coeff": co_p[c].reshape(P, NG * F * GS),
            "ptin": pt[c].reshape(P, NG * F * X),
        })

    res = run_bass_kernel_spmd(nc, in_maps, core_ids=list(range(NCORES)))
    _CACHE["last"] = res

    # host epilogue: extract per-sample 4x4 G/R diagonals, fp64 solve
    G = np.empty((B, C, C), np.float64)
    R = np.empty((B, C, C), np.float64)
    for c in range(NCORES):
        o = np.asarray(res.results[c]["gr_out"], dtype=np.float64)
        for g in range(NG):
            bg = o[:, g * X : g * X + C * GS].reshape(C, GS, C, GS)
            br = o[:, g * X + C * GS : (g + 1) * X].reshape(C, GS, GS, C)
            s0 = c * SPC + g * GS
            G[s0 : s0 + GS] = np.einsum("isjs->sij", bg)
            R[s0 : s0 + GS] = np.einsum("issm->sim", br)

    G = 0.5 * (G + np.swapaxes(G, 1, 2))
    Xs = np.linalg.solve(G, R)
    val = (H * H) * np.einsum("bim,bim->b", R, Xs)
    loss = np.mean((4.0 - val) / 4.0)
    return np.float32(loss)
